# revision 71
# baseline (speedup 1.0000x reference)
"""Additive LoRA adapter (MoE-routed) forward — Trainium2, 8 NeuronCores.

Data-parallel over tokens: each core gets n/8 tokens, weights replicated.
Forward only => no collectives.

Per-core compute (feature-major / "transposed activations" layout):
  - base:   out.T[o,t] += sum_k Wt[k,o-block].T @ xbf[k,t]     (bf16 matmul)
  - router: h[hid,t] = silu(rw1t.T @ xf32 + rb1)               (bf16 matmul)
            logits[t,e] = h[.,t-block].T @ rw2.T               (bf16; rb2+gates
            folded into the exp step of the top-2 softmax)
  - top2 + softmax on VectorE via exp / two maxes / masks
  - coeff.T via PE transpose, expanded to (e,r)-rows via an expand matmul
    that also carries 1/(SA*SB)
  - xa.T[(e,r),t] = At8.T @ x8 (fp8 DoubleRow: 2 contraction chunks/MM)
  - wxa = xa * coeff_expand (DVE, -> fp8e4)
  - delta accumulated into the same PSUM group as base via one fp8
    DoubleRow matmul per output chunk (Bf8, exact scale via SA*SB)

Pipeline: front(t+1) (router/xa/top2/wxa — no W needed) is interleaved
into back(t)'s output-chunk loop so the PE never outruns the x DMA
stream at tile boundaries, and the DVE coeff chain always has matmul
work running beside it. Output stored transposed in bf16 and
un-transposed/upcast on the host.
"""
import sys

sys.path.insert(0, "/opt/trn_rl_repo")

import numpy as np
import ml_dtypes

from concourse import bacc, tile, mybir
from concourse.bass_utils import run_bass_kernel_spmd

N_CORES = 8
D = 2048          # d_in == d_out
E = 16            # populated experts
R = 16            # lora rank
ER = E * R        # 256
HID = 64          # router hidden
P = 128           # partitions
KC = D // P       # 16 contraction chunks
OC = D // P       # 16 output chunks
TT = 512          # token tile
ALPHA = 1.0
WARM = 10         # startup dummy matmuls (cover DMA spin-up, warm HAM)

F32 = mybir.dt.float32
BF16 = mybir.dt.bfloat16
F8 = mybir.dt.float8e4
# fp8 scale plan: at8 = SA*A, bf8 = SB*B, expand = ALPHA/(SA*SB) so the
# LoRA delta lands in the base PSUM at exact scale. SA/SB chosen so every
# fp8 operand sits in e4m3's normal range; 1/(SA*SB) is a power of two so
# the expand matmul adds no rounding error.
SA = 64.0
SB = 8.0
AF = mybir.ActivationFunctionType
ALU = mybir.AluOpType
NEG_BIG = -1.0e30


def _build(n_core: int):
    NT = n_core // TT
    nc = bacc.Bacc("TRN2", target_bir_lowering=False, debug=False,
                   num_devices=N_CORES)

    # x partition-major per tile: [NT, 128, KC, TT] -> one DMA instruction
    # per tile half (16KB contiguous per partition); HWDGE instruction
    # issue (~0.6us each) is the startup bottleneck, so few big DMAs
    xbf_d = nc.dram_tensor("xbf", [NT, P, KC, TT], BF16,
                           kind="ExternalInput").ap()
    xbf8_d = nc.dram_tensor("xbf8", [NT, P, KC, TT], F8,
                            kind="ExternalInput").ap()
    # W.T blocked i-major: [OC, 128i, KC, 128o] -> contiguous 4KB per
    # partition per oc-slab (granular completion for back())
    wt_d = nc.dram_tensor("wt", [OC, P, KC, P], BF16, kind="ExternalInput").ap()
    at_d = nc.dram_tensor("at", [P, KC, ER], F8, kind="ExternalInput").ap()
    bf_d = nc.dram_tensor("bf", [P, 2, D], F8, kind="ExternalInput").ap()
    # router weights fp8 (scaled by SA; undone via silu's scale param)
    rw1t_d = nc.dram_tensor("rw1t", [P, KC, HID], F8,
                            kind="ExternalInput").ap()
    rb1_d = nc.dram_tensor("rb1", [P, 1], F32, kind="ExternalInput").ap()
    # rw2.T replicated into both 64-partition halves (token-split router)
    rw2a_d = nc.dram_tensor("rw2a", [P, E], BF16, kind="ExternalInput").ap()
    # exp(rb2 + gates) broadcast over partitions: folds the expert gate
    # bias into the exp of the top-2 softmax (exp is monotone, so top-2
    # by exp(l)*exp(b) == top-2 by l+b)
    gb_d = nc.dram_tensor("gb", [P, E], F32, kind="ExternalInput").ap()
    bias_d = nc.dram_tensor("biaspp", [P, OC], F32, kind="ExternalInput").ap()
    expand_d = nc.dram_tensor("expand", [E, ER], BF16, kind="ExternalInput").ap()
    ident_d = nc.dram_tensor("ident", [P, P], BF16, kind="ExternalInput").ap()
    outT_d = nc.dram_tensor("outT", [D, n_core], BF16, kind="ExternalOutput").ap()

    with tile.TileContext(nc) as tc:
        with (
            tc.tile_pool(name="const", bufs=1) as constp,
            tc.tile_pool(name="wres", bufs=1) as wres,
            tc.tile_pool(name="xb", bufs=3) as xbp,
            tc.tile_pool(name="xb8", bufs=3) as xbp8,
            tc.tile_pool(name="hp", bufs=2) as hp,
            tc.tile_pool(name="small", bufs=4) as smallp,
            tc.tile_pool(name="cf", bufs=2) as cfp,
            tc.tile_pool(name="outp", bufs=4) as outp,
            tc.tile_pool(name="ps_out", bufs=3, space="PSUM") as ps_out,
            tc.tile_pool(name="ps_xa", bufs=1, space="PSUM") as ps_xa,
            tc.tile_pool(name="ps_h", bufs=1, space="PSUM") as ps_h,
            tc.tile_pool(name="ps_sm", bufs=1, space="PSUM") as ps_sm,
        ):
            # rw1t first (needed by the very first h matmuls)
            rw1t_sb = constp.tile([P, KC, HID], F8)
            nc.sync.dma_start(out=rw1t_sb[:], in_=rw1t_d[:])

            x_tiles = {}
            x8_tiles = {}

            def load_x_tile(tt, halves=1, gate=None):
                xb_sb = xbp.tile([P, KC, TT], BF16, tag="xb")
                if gate is not None:
                    # DMA priority gate: a corner write that reads the last
                    # prologue-critical chunk delays this tile's descriptors
                    # (WAW dep) until the critical stream has landed, so the
                    # SDMA round-robin can't starve tile-0's inputs
                    nc.gpsimd.tensor_copy(xb_sb[0:1, 0, 0:1],
                                          gate[0:1, KC - 1, 0:1])
                hk = KC // halves
                for j in range(halves):
                    nc.sync.dma_start(out=xb_sb[:, j * hk:(j + 1) * hk, :],
                                      in_=xbf_d[tt, :, j * hk:(j + 1) * hk, :])
                x_tiles[tt] = xb_sb

            def load_x8_tile(tt):
                # fp8 copy of the tile for the xa (LoRA) matmuls
                xb8_sb = xbp8.tile([P, KC, TT], F8, tag="xb8")
                nc.sync.dma_start(out=xb8_sb[:], in_=xbf8_d[tt, :, :, :])
                x8_tiles[tt] = xb8_sb

            # front(0) consumes ONLY fp8 inputs (h and xa are DoubleRow),
            # so the fp8 stream loads first; the bf16 x0 (needed from
            # back(0)) streams behind it
            load_x8_tile(0)
            at_sb = wres.tile([P, KC, ER], F8)
            nc.sync.dma_start(out=at_sb[:], in_=at_d[:])

            # ---- small constants (issued after the startup-critical x08) --
            rb1_sb = constp.tile([P, 1], F32)
            nc.sync.dma_start(out=rb1_sb[:], in_=rb1_d[:])
            rw2h_sb = constp.tile([P, E], BF16)
            nc.sync.dma_start(out=rw2h_sb[:], in_=rw2a_d[:])
            gb_sb = constp.tile([P, E], F32)
            nc.sync.dma_start(out=gb_sb[:], in_=gb_d[:])
            bias_sb = constp.tile([P, OC], F32)
            nc.sync.dma_start(out=bias_sb[:], in_=bias_d[:])
            expand_sb = constp.tile([E, ER], BF16)
            nc.sync.dma_start(out=expand_sb[:], in_=expand_d[:])
            ident_sb = constp.tile([P, P], BF16)
            nc.sync.dma_start(out=ident_sb[:], in_=ident_d[:])
            # bf16 x0 behind the fp8 stream; needed from back(0) oc0.
            # x8(1) + bf8 are small and needed early in back(0); the bf16
            # x1 is NOT needed until back(1), so it loads in-loop instead
            load_x_tile(0)
            if NT > 1:
                load_x8_tile(1)
            bf_sb = wres.tile([P, 2, D], F8)
            nc.sync.dma_start(out=bf_sb[:], in_=bf_d[:])
            # W split-gated: the first slabs release early (after the fp8
            # front stream) so back(0) starts with a supply buffer; the
            # rest release once x0 has landed so they never starve it
            wt_sb = wres.tile([P, OC, KC, P], BF16)
            W_EARLY = 6
            nc.gpsimd.tensor_copy(wt_sb[0:1, 0, 0, 0:1],
                                  at_sb[0:1, KC - 1, 0:1])
            for oc in range(W_EARLY):
                nc.sync.dma_start(out=wt_sb[:, oc, :, :],
                                  in_=wt_d[oc, :, :, :])
            nc.gpsimd.tensor_copy(wt_sb[0:1, W_EARLY, 0, 0:1],
                                  x_tiles[0][0:1, KC - 1, 0:1])
            for oc in range(W_EARLY, OC):
                nc.sync.dma_start(out=wt_sb[:, oc, :, :],
                                  in_=wt_d[oc, :, :, :])

            # ---- startup dummy matmuls: cover the DMA spin-up so HAM
            # reaches 2.4GHz before real work arrives ----
            scr_sb = constp.tile([P, TT], BF16)
            nc.vector.memset(scr_sb[:], 1.0)
            warm_ps = ps_out.tile([P, TT], F32, tag="ps", name="ps")
            for _ in range(WARM):
                nc.tensor.matmul(warm_ps[:], lhsT=scr_sb[:, 0:P],
                                 rhs=scr_sb[:], start=True, stop=True)

            def make_front(tt):
                # router + xa + top2 + wxa for tile tt: everything that does
                # NOT need W, split into pieces interleaved into back(tt-1)
                st = {"tt": tt}

                def p_router():
                    # fp8 DoubleRow router matmul; h only drives expert
                    # selection + softmax weights, so fp8 noise is benign
                    xb8_sb = x8_tiles[tt]
                    h_ps = ps_h.tile([HID, TT], F32, tag="h")
                    for k in range(0, KC, 2):
                        nc.tensor.matmul(h_ps[:],
                                         lhsT=rw1t_sb[:, k:k + 2, :],
                                         rhs=xb8_sb[:, k:k + 2, :],
                                         perf_mode=mybir.MatmulPerfMode.DoubleRow,
                                         start=(k == 0), stop=(k == KC - 2))
                    h_sb = hp.tile([HID, TT], BF16)
                    lg_ps = ps_sm.tile([P, TT // P, E], F32, tag="lgct")
                    for s in range(TT // P):
                        nc.scalar.activation(h_sb[:, s * P:(s + 1) * P],
                                             h_ps[:, s * P:(s + 1) * P],
                                             AF.Silu, bias=rb1_sb[0:HID],
                                             scale=1.0 / SA)
                        nc.tensor.matmul(
                            lg_ps[:, s, :], lhsT=h_sb[:, s * P:(s + 1) * P],
                            rhs=rw2h_sb[0:HID, :], start=True, stop=True)
                    st["lg"] = lg_ps

                def p_xa(half):
                    # fp8 DoubleRow: each matmul contracts two 128-row
                    # chunks (virtual 256-row array), halving the MM count
                    xb8_sb = x8_tiles[tt]
                    xp = ps_xa.tile([P, TT], F32, tag=f"xa{half}",
                                    name=f"xa{half}")
                    for k in range(0, KC, 2):
                        nc.tensor.matmul(
                            xp[:],
                            lhsT=at_sb[:, k:k + 2, half * P:(half + 1) * P],
                            rhs=xb8_sb[:, k:k + 2, :],
                            perf_mode=mybir.MatmulPerfMode.DoubleRow,
                            start=(k == 0), stop=(k == KC - 2))
                    st[f"xa{half}"] = xp

                def p_top2():
                    # top2 + softmax -> coeff (token-major), on DVE/ACT
                    lg_ps = st["lg"]
                    coeff_bfs = []
                    for s in range(TT // P):
                        e_raw = smallp.tile([P, E], F32, tag=f"er{s % 2}",
                                            name="e_raw")
                        nc.scalar.activation(e_raw[:], lg_ps[:, s, :], AF.Exp)
                        # fold expert gate bias: e = exp(l) * exp(b)
                        e_sb = smallp.tile([P, E], F32, tag=f"e{s % 2}",
                                           name="e_sb")
                        nc.vector.tensor_tensor(e_sb[:], e_raw[:], gb_sb[:],
                                                op=ALU.mult)
                        m1 = smallp.tile([P, 1], F32, tag="m1")
                        nc.vector.tensor_reduce(m1[:], e_sb[:],
                                                axis=mybir.AxisListType.X,
                                                op=ALU.max)
                        mask1 = smallp.tile([P, E], F32, tag="mask1")
                        nc.vector.tensor_scalar(mask1[:], e_sb[:], m1[:], None,
                                                op0=ALU.is_ge)
                        masked = smallp.tile([P, E], F32, tag="masked")
                        nc.vector.scalar_tensor_tensor(
                            masked[:], in0=mask1[:], scalar=NEG_BIG,
                            in1=e_sb[:], op0=ALU.mult, op1=ALU.add)
                        m2 = smallp.tile([P, 1], F32, tag="m2")
                        nc.vector.tensor_reduce(m2[:], masked[:],
                                                axis=mybir.AxisListType.X,
                                                op=ALU.max)
                        s12 = smallp.tile([P, 1], F32, tag="s12")
                        nc.vector.tensor_tensor(s12[:], m1[:], m2[:],
                                                op=ALU.add)
                        rs = smallp.tile([P, 1], F32, tag="rs")
                        nc.vector.reciprocal(rs[:], s12[:])
                        mask2 = smallp.tile([P, E], F32, tag="mask2")
                        nc.vector.tensor_scalar(mask2[:], e_sb[:], m2[:], None,
                                                op0=ALU.is_ge)
                        coeff_bf = smallp.tile([P, E], BF16,
                                               tag=f"coeff{s % 2}",
                                               name="coeff_bf")
                        nc.vector.scalar_tensor_tensor(
                            coeff_bf[:], in0=e_sb[:], scalar=rs[:],
                            in1=mask2[:], op0=ALU.mult, op1=ALU.mult)
                        coeff_bfs.append(coeff_bf)
                    st["coeffs"] = coeff_bfs

                def p_transp():
                    # PE transpose coeff [128,16] -> [16,128] x4, one bank
                    ct_ps = ps_sm.tile([E, TT // P, P], BF16, tag="lgct",
                                       name="ct_ps")
                    for s in range(TT // P):
                        nc.tensor.transpose(ct_ps[:, s, :], st["coeffs"][s][:],
                                            ident_sb[:])
                    coefft_sb = cfp.tile([E, TT], BF16, tag="coefft")
                    nc.vector.tensor_copy(coefft_sb[:], ct_ps[:])
                    st["coefft"] = coefft_sb

                def p_wxa():
                    # expand coeff.T rows to (e,r) rows; wxa = xa * cexp
                    # (cexp carries coeff/(SA*SB)); fp8 out for DoubleRow
                    wxa_sb = cfp.tile([P, 2, TT], F8, tag="wxa")
                    for half in range(2):
                        cx_ps = ps_sm.tile([P, TT], F32, tag="cx",
                                           name="cx_ps")
                        nc.tensor.matmul(
                            cx_ps[:],
                            lhsT=expand_sb[:, half * P:(half + 1) * P],
                            rhs=st["coefft"][:], start=True, stop=True)
                        cx_sb = cfp.tile([P, TT], F32, tag=f"cxs{half}",
                                         name="cx_sb")
                        nc.vector.tensor_copy(cx_sb[:], cx_ps[:])
                        nc.vector.tensor_tensor(wxa_sb[:, half, :],
                                                st[f"xa{half}"][:],
                                                cx_sb[:], op=ALU.mult)
                    st["wxa"] = wxa_sb

                pieces = [p_router, lambda: p_xa(0), lambda: p_xa(1),
                          p_top2, p_transp, p_wxa]
                return st, pieces

            def back_oc(tt, st, oc):
                # base + delta accumulated for one 128-row output chunk
                t0 = tt * TT
                xb_sb, wxa_sb = x_tiles[tt], st["wxa"]
                ps = ps_out.tile([P, TT], F32)
                for k in range(KC):
                    nc.tensor.matmul(ps[:], lhsT=wt_sb[:, oc, k, :],
                                     rhs=xb_sb[:, k, :],
                                     start=(k == 0), stop=False)
                nc.tensor.matmul(ps[:],
                                 lhsT=bf_sb[:, 0:2, oc * P:(oc + 1) * P],
                                 rhs=wxa_sb[:, 0:2, :],
                                 perf_mode=mybir.MatmulPerfMode.DoubleRow,
                                 start=False, stop=True)
                o_sb = outp.tile([P, TT], BF16)
                # epilogue (bias add + bf16 cast) on DVE: no ACT table
                # thrash, ACT keeps only silu/exp
                nc.vector.tensor_scalar(o_sb[:], ps[:], bias_sb[:, oc:oc + 1],
                                        None, op0=ALU.add)
                nc.sync.dma_start(
                    out=outT_d[oc * P:(oc + 1) * P, t0:t0 + TT],
                    in_=o_sb[:])

            # front(t+1) pieces slotted into back(t)'s oc loop: PE always
            # has W-independent work while wt/xb stream in, and the DVE
            # coeff chain for t+1 runs beside back(t)'s matmuls.
            SLOT = {1: 0, 2: 1, 4: 2, 7: 3, 9: 4, 12: 5}
            st0, pieces0 = make_front(0)
            for p in pieces0:
                p()
            states = {0: st0}
            for tt in range(NT):
                st = states.pop(tt)
                if tt + 1 < NT:
                    nst, pieces = make_front(tt + 1)
                    states[tt + 1] = nst
                else:
                    pieces = []
                for oc in range(OC):
                    back_oc(tt, st, oc)
                    if oc == 8:
                        # bf16 x(t+1) is only consumed by back(t+1); fp8
                        # x8(t+2) only by front(t+2) — both have half a
                        # back() of slack, so they stream in-loop without
                        # stealing bandwidth from W's critical first slabs
                        if tt + 1 < NT:
                            load_x_tile(tt + 1)
                        if tt + 2 < NT:
                            load_x8_tile(tt + 2)
                    if oc in SLOT and pieces:
                        pieces[SLOT[oc]]()

    nc.compile()
    return nc


_CACHE = {}


def _get_nc(n_core: int):
    if n_core not in _CACHE:
        _CACHE[n_core] = _build(n_core)
    return _CACHE[n_core]


def _prep_in_maps(x, W, bias, rw1, rb1, rw2, rb2, A, B, gates):
    x, W, bias, rw1, rb1, rw2, rb2, A, B, gates = (
        np.asarray(v) for v in (x, W, bias, rw1, rb1, rw2, rb2, A, B, gates))
    xf = np.ascontiguousarray(x.reshape(-1, D).astype(np.float32))
    n = xf.shape[0]
    assert n % N_CORES == 0
    n_core = n // N_CORES

    bf16 = ml_dtypes.bfloat16
    xTb = np.ascontiguousarray(xf.T).astype(bf16)        # [D, n] bf16
    # W.T blocked i-major [OC, 128i, KC, 128o]
    wt = np.ascontiguousarray(
        W.astype(np.float32).T.reshape(KC, P, OC, P).transpose(2, 1, 0, 3)
    ).astype(bf16)
    f8 = ml_dtypes.float8_e4m3
    # partition-major packs: one DMA instruction each on device.
    # A scaled by SA and B by SB into e4m3's normal range (delta exactness
    # restored via the expand constant below).
    at = np.ascontiguousarray(
        (A.astype(np.float32) * SA).reshape(ER, D).T.reshape(KC, P, ER)
        .transpose(1, 0, 2)).astype(f8)
    bfl = np.ascontiguousarray(
        (B.astype(np.float32) * SB).transpose(0, 2, 1).reshape(2, P, D)
        .transpose(1, 0, 2)).astype(f8)
    rw1t = np.ascontiguousarray(
        (rw1.astype(np.float32) * SA).T.reshape(KC, P, HID)
        .transpose(1, 0, 2)).astype(f8)
    # router consts replicated into both 64-partition halves
    rb1c = np.ascontiguousarray(
        np.tile(rb1.astype(np.float32).reshape(HID, 1), (2, 1)))
    rw2a = np.ascontiguousarray(
        np.tile(rw2[:E].astype(np.float32).T, (2, 1))).astype(bf16)
    # exp of the expert gate bias, broadcast over partitions
    gb = np.ascontiguousarray(
        np.broadcast_to(
            np.exp(rb2[:E].astype(np.float64)
                   + gates.astype(np.float64)).astype(np.float32),
            (P, E)).copy())
    biaspp = np.ascontiguousarray(
        bias.astype(np.float32).reshape(OC, P).T)
    expand = np.zeros((E, ER), np.float32)
    for e in range(E):
        expand[e, e * R:(e + 1) * R] = ALPHA / (SA * SB)
    expand = expand.astype(bf16)
    ident = np.eye(P, dtype=np.float32).astype(bf16)

    shared = {"wt": wt, "at": at, "bf": bfl, "rw1t": rw1t, "rb1": rb1c,
              "rw2a": rw2a, "gb": gb, "biaspp": biaspp, "expand": expand,
              "ident": ident}
    NT = n_core // TT
    in_maps = []
    for c in range(N_CORES):
        sl = slice(c * n_core, (c + 1) * n_core)
        xc = (xTb[:, sl].reshape(KC, P, NT, TT)
              .transpose(2, 1, 0, 3))
        xc = np.ascontiguousarray(xc)
        in_maps.append({"xbf": xc, "xbf8": xc.astype(f8), **shared})
    return in_maps, n_core


def _core_out(result_map):
    # per-core unshard: kernel emits the output transposed bf16 [D, n_core]
    return np.asarray(result_map["outT"]).astype(np.float32).T


def kernel(x, W, bias, rw1, rb1, rw2, rb2, A, B, gates):
    lead = x.shape[:-1]
    in_maps, n_core = _prep_in_maps(x, W, bias, rw1, rb1, rw2, rb2, A, B,
                                    gates)
    n = n_core * N_CORES
    nc = _get_nc(n_core)
    res = None
    for attempt in range(3):
        try:
            res = run_bass_kernel_spmd(nc, in_maps,
                                       core_ids=list(range(N_CORES)))
            break
        except Exception:
            # sporadic NRT_EXEC_UNIT_UNRECOVERABLE on a fresh NEFF; retry
            if attempt == 2:
                raise
            import time as _time

            _time.sleep(10)

    out = np.empty((n, D), np.float32)
    for c in range(N_CORES):
        out[c * n_core:(c + 1) * n_core] = _core_out(res.results[c])
    return out.reshape(*lead, D)
